# revision 1
# baseline (speedup 1.0000x reference)
"""Trainium2 Bass kernel for SAM-style decomposed rel-pos attention (v3).

Problem: B=1, HW=2304 (48x48), NH=16 heads, DH=64, D=1024, f32 in/out.
  attn = softmax(q*scale @ k^T + rel_h[qh,kh] + rel_w[qw,kw]); out = attn @ v

Strategy (8 NeuronCores, SPMD): 2 heads per core. Key ideas:
- rel_h is folded into the single score matmul per k-tile (one-hot Eh rows
  + K^T stacked as the stationary operand; gathered rel_h^T rows + Q^T as
  the moving operand) -> 18 score matmuls per (head, chunk) instead of 36.
- rel_w is applied MULTIPLICATIVELY after exp: P = exp(S_qk+relh) * Ew
  where Ew[k,q] = exp(rel_w^T[kw(k), q]). kw(k) is periodic with period 48
  and 128 = 2*48 + 32, so only 3 row-rotations (offsets 0/32/16 = kt mod 3)
  of exp_relw exist -> a [128, 3, HW] "patterns" tile serves every k-tile
  triple via one DVE tensor_mul per 3-k-tile group.
- Diagonal gathers (rel tables are banded) are ONE DMA each via a DRAM
  roundtrip: T1 tables stored to scratch DRAM, re-loaded with a 3D access
  pattern whose middle dim strides -2256 (one row up, 48 cols right).
- exp on ScalarE in [128, 3, qn] groups from PSUM; PV matmuls run PV_LAG
  groups behind the score matmuls; the reciprocal-broadcast matmul of each
  chunk's epilogue is deferred further so the in-order PE stream never
  waits -> PE_HAM un-throttles the clock 1.2 -> 2.4 GHz.
- Head 1's prep (table matmuls, stores, gathers, exp, patterns) is
  interleaved into head 0's main loop at chunk boundaries so the PE and
  DMA rings stay busy; head 0's prep is the only exposed startup.
- Softmax denominator: ones-column in V_aug -> row 64 of the PV output;
  reciprocal on a [128, 4] transposed view (two tiny DMAs) since a
  [1, 512] single-partition reciprocal wastes 127 DVE lanes.
"""

import sys

sys.path.insert(0, "/opt/trn_rl_repo")

import numpy as np
import ml_dtypes

from concourse import bacc, mybir, tile
from concourse.tile import add_dep_helper
from concourse.bass_utils import run_bass_kernel_spmd

BF16 = mybir.dt.bfloat16
F32 = mybir.dt.float32
BF = ml_dtypes.bfloat16

H = 48
W = 48
HW = H * W          # 2304
DH = 64
NH = 16
N_CORES = 8
HPC = 2             # heads per core
KT = HW // 128      # 18 k tiles
QCHUNKS = [(0, 512), (512, 512), (1024, 512), (1536, 512), (2048, 256)]
NG = KT // 3        # 6 groups of 3 k-tiles
PV_LAG = 4          # PV runs this many groups behind the score matmuls
RB_LAG = 7          # reciprocal-broadcast matmul deferral (groups)

_NC = None


def _build_nc():
    nc = bacc.Bacc(None, target_bir_lowering=False)

    lhs_p = nc.dram_tensor("lhs_p", [128, HPC * HW], BF16, kind="ExternalInput")
    rq_p = nc.dram_tensor("rq_p", [128, HPC * 2 * HW], BF16, kind="ExternalInput")
    v_til = nc.dram_tensor("v_til", [128, HPC * KT * 65], BF16, kind="ExternalInput")
    rhv = nc.dram_tensor("rhv", [64, 95], BF16, kind="ExternalInput")
    rwv = nc.dram_tensor("rwv", [64, 95], BF16, kind="ExternalInput")
    out_t = nc.dram_tensor("out_t", [128, HW], F32, kind="ExternalOutput")
    t1d = [nc.dram_tensor(f"t1d{h}", [95, HW], BF16, kind="Internal") for h in range(HPC)]
    t2d = [nc.dram_tensor(f"t2d{h}", [95, HW], BF16, kind="Internal") for h in range(HPC)]

    Exp = mybir.ActivationFunctionType.Exp

    with tile.TileContext(nc) as tc:
        with (
            tc.tile_pool(name="const", bufs=1) as cpool,
            tc.tile_pool(name="stack", bufs=2) as spool,
            tc.tile_pool(name="p1t", bufs=4) as p1pool,
            tc.tile_pool(name="p2t", bufs=6) as p2pool,
            tc.tile_pool(name="epil", bufs=3) as epool,
            tc.tile_pool(name="ps_s", bufs=2, space="PSUM") as ps_s,
            tc.tile_pool(name="ps_o", bufs=2, space="PSUM") as ps_o,
        ):
            rhv_sb = cpool.tile([128, 95], BF16, tag="rhv")
            rwv_sb = cpool.tile([128, 95], BF16, tag="rwv")
            ones1 = cpool.tile([1, 64], BF16, tag="ones1")
            nc.sync.dma_start(rhv_sb[64:128, :], rhv[:, :])
            nc.sync.dma_start(rwv_sb[64:128, :], rwv[:, :])
            nc.gpsimd.memset(ones1[:], 1.0)

            # per-head state dicts
            hs = [dict() for _ in range(HPC)]

            def ph1_load(hh):
                dmae = nc.sync if hh == 0 else nc.gpsimd
                c0, c1 = hh * 64, (hh + 1) * 64
                s = hs[hh]
                s["c0"], s["c1"] = c0, c1
                s["lhsT"] = spool.tile([128, HW], BF16, tag="lhsT", name=f"lhsT{hh}")
                # rq[:, 0, :] = score moving stack (relh rows land in 0:48,
                # zeros 48:64, Q^T 64:128); rq[:, 1, :] = w-major Q^T
                s["rq"] = spool.tile([128, 2, HW], BF16, tag="rq", name=f"rq{hh}")
                s["vt"] = spool.tile([128, KT * 65], BF16, tag="vt", name=f"vt{hh}")
                s["t12"] = spool.tile([95, 2, HW], BF16, tag="t12", name=f"t12{hh}")
                s["relw"] = spool.tile([48, HW], BF16, tag="relw", name=f"relw{hh}")
                s["expw"] = spool.tile([48, HW], BF16, tag="expw", name=f"expw{hh}")
                s["pats"] = spool.tile([128, 3, HW], BF16, tag="pats", name=f"pats{hh}")
                for (l0, ln) in ((0, 768), (768, 768), (1536, 768)):
                    dmae.dma_start(
                        s["rq"][:, :, l0 : l0 + ln],
                        rq_p[:, 2 * hh * HW : 2 * (hh + 1) * HW].rearrange(
                            "p (a b) -> p a b", b=HW
                        )[:, :, l0 : l0 + ln],
                    )
                dmae.dma_start(
                    s["lhsT"][:, :], lhs_p[:, hh * HW : (hh + 1) * HW]
                )
                dmae.dma_start(s["vt"][:, :], v_til[:, hh * KT * 65 : (hh + 1) * KT * 65])

            # gather piece ranges: piece c covers blocks [lo, hi) whose cols
            # fit inside table-store chunks 0..c (512 cols each)
            PIECES = [(0, 10), (10, 21), (21, 32), (32, 42), (42, 48)]

            def ph1_tabs(hh, lo, hi):
                # per chunk: T2w (w-major rel_w table) and T1h (rel_h table)
                # matmuls -> one fused cast -> chunked DRAM stores (ring A)
                # -> gather pieces (ring B: never stuck behind a store).
                # t12[:, 0, :] = T2w, t12[:, 1, :] = T1h.
                s = hs[hh]
                ringA = nc.sync if hh == 0 else nc.gpsimd
                ringB = nc.gpsimd if hh == 0 else nc.sync
                for ci in range(lo, hi):
                    (q0, qn) = QCHUNKS[ci]
                    tp = ps_s.tile([128, 3, 512], F32, tag="s", name=f"tp{hh}_{ci}")
                    nc.tensor.matmul(
                        tp[0:95, 0, 0:qn], rwv_sb[64:128, :],
                        s["rq"][64:128, 1, q0 : q0 + qn], start=True, stop=True,
                    )
                    nc.tensor.matmul(
                        tp[0:95, 1, 0:qn], rhv_sb[64:128, :],
                        s["rq"][64:128, 0, q0 : q0 + qn], start=True, stop=True,
                    )
                    nc.vector.tensor_copy(
                        s["t12"][:, :, q0 : q0 + qn], tp[0:95, 0:2, 0:qn]
                    )
                    for tbl, dram in ((0, t2d[hh]), (1, t1d[hh])):
                        ringA.dma_start(
                            dram[0:95, q0 : q0 + qn], s["t12"][0:95, tbl, q0 : q0 + qn]
                        )
                    # gather pieces (diagonal re-load)
                    (ba, bb) = PIECES[ci]
                    nbl = bb - ba
                    # rel_h: rq[j, 0, (h,w)] = T1h[47-h+j, 48h+w], h in [ba, bb)
                    dsth = s["rq"][0:48, 0, 48 * ba : 48 * bb].rearrange(
                        "p (h w) -> p h w", w=48
                    )
                    srch = t1d[hh][47 - ba : 95 - ba, 48 * ba : 48 * bb].rearrange(
                        "j (h w) -> j h w", w=48
                    )
                    srch.ap[1] = [-2256, nbl]
                    ringB.dma_start(dsth, srch)
                    # rel_w (w-major): relw[j, (w,h)] = T2w[47-w+j, 48w+h]
                    dstw = s["relw"][0:48, 48 * ba : 48 * bb].rearrange(
                        "p (w h) -> p w h", h=48
                    )
                    srcw = t2d[hh][47 - ba : 95 - ba, 48 * ba : 48 * bb].rearrange(
                        "j (w h) -> j w h", h=48
                    )
                    srcw.ap[1] = [-2256, nbl]
                    ringB.dma_start(dstw, srcw)

            def ph1_relw_tail(hh, half):
                # exp with an un-permuting (w-major -> q-major) input AP
                # half=None: full width in one pass (fewer DMAs; for the
                # latency-hidden head)
                s = hs[hh]
                if half is None:
                    h0, co, cn = 0, 0, HW
                    nhb = 48
                else:
                    h0 = half * 24  # h-blocks 0:24/24:48 -> cols 0:1152/1152:2304
                    co, cn = h0 * 48, 1152
                    nhb = 24
                nc.scalar.activation(
                    s["expw"][:, co : co + cn].rearrange("p (h w) -> p h w", w=48),
                    s["relw"][:, :].rearrange("p (w h) -> p h w", w=48)[:, h0 : h0 + nhb, :],
                    Exp,
                )
                ndma = 0
                for j, off in enumerate((0, 32, 16)):
                    p = 0
                    while p < 128:
                        r0 = (p + off) % 48
                        n = min(48 - r0, 128 - p)
                        eng = (nc.sync, nc.gpsimd)[(hh + ndma) % 2]
                        eng.dma_start(
                            s["pats"][p : p + n, j, co : co + cn],
                            s["expw"][r0 : r0 + n, co : co + cn],
                        )
                        p += n
                        ndma += 1

            # ---- main loop machinery (per head) ----
            def make_main(hh):
                s = hs[hh]
                st = {"pend": [], "epiB": [], "g": 0}

                def epilogue_a(ci, q0, qn, o_ps):
                    den = epool.tile([1, 512], F32, tag="den", name=f"den{hh}_{ci}")
                    nc.vector.tensor_copy(den[0:1, 0:qn], o_ps[64:65, 0:qn])
                    dmae = nc.sync if ci % 2 == 0 else nc.gpsimd
                    dent = epool.tile([128, 4], F32, tag="dent", name=f"dent{hh}_{ci}")
                    nr = qn // 128
                    dmae.dma_start(dent[0:128, 0:nr], den[0:1, 0:qn])
                    rect = epool.tile([128, 4], F32, tag="rect", name=f"rect{hh}_{ci}")
                    nc.vector.reciprocal(rect[0:128, 0:nr], dent[0:128, 0:nr])
                    recb = epool.tile([128, 4], BF16, tag="recb", name=f"recb{hh}_{ci}")
                    nc.vector.tensor_copy(recb[0:128, 0:nr], rect[0:128, 0:nr])
                    recr = epool.tile([1, 512], BF16, tag="recr", name=f"recr{hh}_{ci}")
                    dmae.dma_start(recr[0:1, 0:qn], recb[0:128, 0:nr])
                    return recr

                def epilogue_b(ci, q0, qn, o_ps, recr):
                    rb_ps = ps_s.tile([128, 3, 512], F32, tag="s", name=f"rb{hh}_{ci}")
                    nc.tensor.matmul(
                        rb_ps[0:64, 0, 0:qn], ones1[:], recr[0:1, 0:qn],
                        start=True, stop=True,
                    )
                    rb_sb = epool.tile([64, 512], F32, tag="rb_sb", name=f"rbs{hh}_{ci}")
                    nc.vector.tensor_copy(rb_sb[:, 0:qn], rb_ps[0:64, 0, 0:qn])
                    ot = epool.tile([64, 512], F32, tag="ot", name=f"ot{hh}_{ci}")
                    nc.vector.tensor_mul(
                        ot[:, 0:qn], o_ps[0:64, 0:qn], rb_sb[:, 0:qn]
                    )
                    nc.sync.dma_start(out_t[s["c0"] : s["c1"], q0 : q0 + qn], ot[:, 0:qn])

                def flush(last_mm, keep):
                    while len(st["pend"]) > keep:
                        (ci, q0, qn, o_ps, g, p2) = st["pend"].pop(0)
                        for j in range(3):
                            kt = 3 * g + j
                            pv = nc.tensor.matmul(
                                o_ps[0:65, 0:qn],
                                s["vt"][:, kt * 65 : (kt + 1) * 65],
                                p2[:, j, 0:qn],
                                start=(kt == 0), stop=(kt == KT - 1),
                            )
                            if last_mm is not None:
                                add_dep_helper(pv.ins, last_mm.ins, sync=False,
                                               reason="pv after score mms")
                        if g == NG - 1:
                            recr = epilogue_a(ci, q0, qn, o_ps)
                            st["epiB"].append((ci, q0, qn, o_ps, recr, st["g"] + RB_LAG))
                    while st["epiB"] and st["epiB"][0][5] <= st["g"]:
                        (ci, q0, qn, o_ps, recr, _) = st["epiB"].pop(0)
                        epilogue_b(ci, q0, qn, o_ps, recr)

                def chunk(ci):
                    (q0, qn) = QCHUNKS[ci]
                    o_ps = ps_o.tile([65, 512], F32, tag="o", name=f"o{hh}_{ci}")
                    for g in range(NG):
                        s_ps = ps_s.tile([128, 3, 512], F32, tag="s",
                                         name=f"s{hh}_{ci}_{g}")
                        last_mm = None
                        for j in range(3):
                            kt = 3 * g + j
                            last_mm = nc.tensor.matmul(
                                s_ps[:, j, 0:qn],
                                s["lhsT"][:, kt * 128 : (kt + 1) * 128],
                                s["rq"][:, 0, q0 : q0 + qn],
                                start=True, stop=True,
                            )
                        st["g"] += 1
                        flush(last_mm, PV_LAG - 1)
                        p1 = p1pool.tile([128, 3, 512], BF16, tag="p1")
                        nc.scalar.activation(p1[:, :, 0:qn], s_ps[:, :, 0:qn], Exp)
                        p2 = p2pool.tile([128, 3, 512], BF16, tag="p2")
                        nc.vector.tensor_mul(
                            p2[:, :, 0:qn], p1[:, :, 0:qn],
                            s["pats"][:, :, q0 : q0 + qn],
                        )
                        st["pend"].append((ci, q0, qn, o_ps, g, p2))

                def finish_pvs():
                    st["g"] += PV_LAG
                    flush(None, 0)

                def drain(n=1):
                    for _ in range(n):
                        if st["epiB"]:
                            (ci, q0, qn, o_ps, recr, _) = st["epiB"].pop(0)
                            epilogue_b(ci, q0, qn, o_ps, recr)

                return chunk, finish_pvs, drain

            # ---- schedule ----
            ph1_load(0)
            ph1_load(1)
            ph1_tabs(0, 0, 5)
            ph1_relw_tail(0, 0)
            ph1_relw_tail(0, 1)
            chunk0, finish_pvs0, drain0 = make_main(0)
            chunk1, finish_pvs1, drain1 = make_main(1)
            chunk0(0)
            ph1_tabs(1, 0, 3)
            chunk0(1)
            ph1_tabs(1, 3, 5)
            chunk0(2)
            ph1_relw_tail(1, 0)
            ph1_relw_tail(1, 1)
            chunk0(3)
            chunk0(4)
            finish_pvs0()
            chunk1(0)
            drain0(2)
            chunk1(1)
            drain0(2)
            chunk1(2)
            drain0(2)
            chunk1(3)
            chunk1(4)
            finish_pvs1()
            drain1(5)

    nc.compile()
    return nc


def _get_nc():
    global _NC
    if _NC is None:
        _NC = _build_nc()
    return _NC


def _host_prep(q, k, v, rel_pos_h, rel_pos_w):
    q2 = np.asarray(q, np.float32).reshape(HW, NH * DH)
    k2 = np.asarray(k, np.float32).reshape(HW, NH * DH)
    v2 = np.asarray(v, np.float32).reshape(HW, NH * DH)
    rph = np.asarray(rel_pos_h, np.float32)
    rpw = np.asarray(rel_pos_w, np.float32)

    ar = np.arange(48)
    # reversed rel tables, x8 cancels the 0.125 q scale
    rhv = np.ascontiguousarray((8.0 * rph[::-1]).T).astype(BF)   # (64, 95)
    rwv = np.ascontiguousarray((8.0 * rpw[::-1]).T).astype(BF)
    kk = np.arange(HW)
    eh = np.zeros((64, HW), np.float32)
    eh[:48] = kk[None, :] // 48 == ar[:, None]
    eh = eh.astype(BF)

    onecol = np.ones((HW, 1), np.float32)
    in_maps = []
    for c in range(N_CORES):
        sl = slice(c * 128, (c + 1) * 128)
        qs = (q2[:, sl].T * 0.125).astype(BF)                    # (128, HW)
        qw = np.ascontiguousarray(
            qs.reshape(128, 48, 48).transpose(0, 2, 1)
        ).reshape(128, HW)                                       # w-major cols
        ks = k2[:, sl].T.astype(BF)
        lhs_p = np.zeros((128, HPC, HW), BF)
        rq_p = np.zeros((128, HPC, 2, HW), BF)
        vparts = []
        for hh in range(HPC):
            r0, r1 = hh * 64, (hh + 1) * 64
            lhs_p[0:64, hh, :] = eh
            lhs_p[64:128, hh, :] = ks[r0:r1]
            rq_p[64:128, hh, 0, :] = qs[r0:r1]
            rq_p[64:128, hh, 1, :] = qw[r0:r1]
            vh = v2[:, c * 128 + hh * 64 : c * 128 + (hh + 1) * 64]
            va = np.concatenate([vh, onecol], axis=1)            # (HW, 65)
            vparts.append(va.reshape(KT, 128, 65).transpose(1, 0, 2).reshape(128, KT * 65))
        v_til = np.concatenate(vparts, axis=1).astype(BF)        # (128, 2*18*65)
        in_maps.append(
            dict(
                lhs_p=lhs_p.reshape(128, HPC * HW),
                rq_p=rq_p.reshape(128, HPC * 2 * HW),
                v_til=v_til, rhv=rhv, rwv=rwv,
            )
        )
    return in_maps


def _assemble(results):
    cols = [np.asarray(r["out_t"], np.float32).T for r in results]  # (HW, 128) each
    return np.concatenate(cols, axis=1).reshape(1, H, W, NH * DH)


def kernel(q, k, v, rel_pos_h, rel_pos_w):
    nc = _get_nc()
    in_maps = _host_prep(q, k, v, rel_pos_h, rel_pos_w)
    res = run_bass_kernel_spmd(nc, in_maps, core_ids=list(range(N_CORES)))
    return _assemble(res.results)



# revision 5
# speedup vs baseline: 1.0132x; 1.0132x over previous
"""Trainium2 Bass kernel for SAM-style decomposed rel-pos attention (v4).

Problem: B=1, HW=2304 (48x48), NH=16 heads, DH=64, D=1024, f32 in/out.
  attn = softmax(q*scale @ k^T + rel_h[qh,kh] + rel_w[qw,kw]); out = attn @ v

Strategy (8 NeuronCores, SPMD): 2 heads per core. Key ideas (v3 base):
- rel_h folded into the score matmul (one-hot Eh rows + K^T stationary;
  gathered rel_h rows + Q^T moving) -> 18 score matmuls per (head, chunk).
- rel_w applied multiplicatively after exp: P = exp(S_qk+relh) * Ew with
  only 3 row-rotations of exp_relw -> [128, 3, HW] patterns tile.
- Diagonal gathers via scratch-DRAM roundtrip with a -2256-stride AP.
- exp on ScalarE in [128, 3, qn] groups; PV matmuls lag PV_LAG groups.

v4 scheduling changes (same math):
- Startup pipelining: per-table split of the prep chain. The T2w chain
  (mm -> cast -> store -> gather -> exp -> pats) is the critical path to
  the first chunk; T1h pieces 0-1 are the only rel_h prerequisites for
  chunk 0. Everything else (T1h pieces 2-4, the second exp half, the
  second pats half, and ALL of head 1's prep) is interleaved into the
  main loop at GROUP granularity via an insert queue.
- Input DMA diet: rq_p [128,2,HW] -> rqs_p [80,HW] (16 zero rows + 64
  q-major Q^T rows) + rqw_p [64,HW] (w-major, lands first; unblocks the
  T2w matmuls ~1.5us after the preamble).
- Epilogue: reciprocal straight off the PSUM denominator row [1, qn]
  (drops the two transpose DMAs + a copy), cast, broadcast matmul,
  multiply reading both PSUM operands. RB_LAG 7 -> 4 so head-1's
  epilogues drain inside the main loop instead of piling at the end.
"""

import sys

sys.path.insert(0, "/opt/trn_rl_repo")

import numpy as np
import ml_dtypes

from concourse import bacc, mybir, tile
from concourse.tile import add_dep_helper
from concourse.bass_utils import run_bass_kernel_spmd

BF16 = mybir.dt.bfloat16
F32 = mybir.dt.float32
BF = ml_dtypes.bfloat16

H = 48
W = 48
HW = H * W          # 2304
DH = 64
NH = 16
N_CORES = 8
HPC = 2             # heads per core
KT = HW // 128      # 18 k tiles
QCHUNKS = [(0, 512), (512, 512), (1024, 512), (1536, 512), (2048, 256)]
NG = KT // 3        # 6 groups of 3 k-tiles
PV_LAG = 4          # PV runs this many groups behind the score matmuls
RB_LAG = 4          # reciprocal-broadcast matmul deferral (groups)

# gather piece ranges: piece c covers blocks [lo, hi) whose table cols
# fit inside table-store chunks 0..c (QCHUNKS cols)
PIECES = [(0, 10), (10, 21), (21, 32), (32, 42), (42, 48)]

_NC = None


def _build_nc():
    nc = bacc.Bacc(None, target_bir_lowering=False)

    lhs_p = nc.dram_tensor("lhs_p", [128, HPC * HW], BF16, kind="ExternalInput")
    rqs_p = nc.dram_tensor("rqs_p", [80, HPC * HW], BF16, kind="ExternalInput")
    rqw_p = nc.dram_tensor("rqw_p", [64, HPC * HW], BF16, kind="ExternalInput")
    v_til = nc.dram_tensor("v_til", [128, HPC * KT * 65], BF16, kind="ExternalInput")
    rhv = nc.dram_tensor("rhv", [64, 95], BF16, kind="ExternalInput")
    rwv = nc.dram_tensor("rwv", [64, 95], BF16, kind="ExternalInput")
    out_t = nc.dram_tensor("out_t", [128, HW], F32, kind="ExternalOutput")
    t1d = [nc.dram_tensor(f"t1d{h}", [95, HW], BF16, kind="Internal") for h in range(HPC)]
    t2d = [nc.dram_tensor(f"t2d{h}", [95, HW], BF16, kind="Internal") for h in range(HPC)]

    Exp = mybir.ActivationFunctionType.Exp

    with tile.TileContext(nc) as tc:
        with (
            tc.tile_pool(name="const", bufs=1) as cpool,
            tc.tile_pool(name="stack", bufs=2) as spool,
            tc.tile_pool(name="p1t", bufs=4) as p1pool,
            tc.tile_pool(name="p2t", bufs=6) as p2pool,
            tc.tile_pool(name="epil", bufs=3) as epool,
            tc.tile_pool(name="ps_s", bufs=2, space="PSUM") as ps_s,
            tc.tile_pool(name="ps_o", bufs=2, space="PSUM") as ps_o,
        ):
            # rel tables on both partition halves: rows 0:64 feed the T2w
            # matmuls (rqw lives on partitions 0:64), rows 64:128 feed T1h
            # (q-major Q^T lives on rq partitions 64:128).
            rhv_sb = cpool.tile([128, 95], BF16, tag="rhv")
            rwv_sb = cpool.tile([128, 95], BF16, tag="rwv")
            ones1 = cpool.tile([1, 64], BF16, tag="ones1")
            nc.sync.dma_start(rwv_sb[0:64, :], rwv[:, :])
            nc.sync.dma_start(rhv_sb[64:128, :], rhv[:, :])
            nc.gpsimd.memset(ones1[:], 1.0)

            hs = [dict() for _ in range(HPC)]

            def ph1_load(hh):
                dmae = nc.sync if hh == 0 else nc.gpsimd
                c0, c1 = hh * 64, (hh + 1) * 64
                s = hs[hh]
                s["c0"], s["c1"] = c0, c1
                s["lhsT"] = spool.tile([128, HW], BF16, tag="lhsT", name=f"lhsT{hh}")
                s["rq"] = spool.tile([128, HW], BF16, tag="rq", name=f"rq{hh}")
                s["rqw"] = spool.tile([64, HW], BF16, tag="rqw", name=f"rqw{hh}")
                s["vt"] = spool.tile([128, KT * 65], BF16, tag="vt", name=f"vt{hh}")
                s["t1sb"] = spool.tile([95, HW], BF16, tag="t1sb", name=f"t1sb{hh}")
                s["t2sb"] = spool.tile([95, HW], BF16, tag="t2sb", name=f"t2sb{hh}")
                s["relw"] = spool.tile([48, HW], BF16, tag="relw", name=f"relw{hh}")
                s["expw"] = spool.tile([48, HW], BF16, tag="expw", name=f"expw{hh}")
                s["pats"] = spool.tile([128, 3, HW], BF16, tag="pats", name=f"pats{hh}")
                # w-major Q first: it unblocks the T2w (pats-critical) chain
                dmae.dma_start(s["rqw"][:, :], rqw_p[:, hh * HW : (hh + 1) * HW])
                dmae.dma_start(s["rq"][48:128, :], rqs_p[:, hh * HW : (hh + 1) * HW])
                dmae.dma_start(s["lhsT"][:, :], lhs_p[:, hh * HW : (hh + 1) * HW])
                dmae.dma_start(s["vt"][:, :], v_til[:, hh * KT * 65 : (hh + 1) * KT * 65])

            def tabw(hh, ci):
                # T2w chunk: matmul -> cast -> DRAM store -> diagonal gather
                s = hs[hh]
                ringA = nc.sync if hh == 0 else nc.gpsimd
                ringB = nc.gpsimd if hh == 0 else nc.sync
                (q0, qn) = QCHUNKS[ci]
                tp = ps_s.tile([128, 3, 512], F32, tag="s", name=f"tpw{hh}_{ci}")
                nc.tensor.matmul(
                    tp[0:95, 0, 0:qn], rwv_sb[0:64, :],
                    s["rqw"][:, q0 : q0 + qn], start=True, stop=True,
                )
                nc.vector.tensor_copy(s["t2sb"][:, q0 : q0 + qn], tp[0:95, 0, 0:qn])
                ringA.dma_start(
                    t2d[hh][0:95, q0 : q0 + qn], s["t2sb"][0:95, q0 : q0 + qn]
                )
                (ba, bb) = PIECES[ci]
                nbl = bb - ba
                # rel_w (w-major): relw[j, (w,h)] = T2w[47-w+j, 48w+h]
                dstw = s["relw"][0:48, 48 * ba : 48 * bb].rearrange(
                    "p (w h) -> p w h", h=48
                )
                srcw = t2d[hh][47 - ba : 95 - ba, 48 * ba : 48 * bb].rearrange(
                    "j (w h) -> j w h", h=48
                )
                srcw.ap[1] = [-2256, nbl]
                ringB.dma_start(dstw, srcw)

            def tabh(hh, ci):
                # T1h chunk: matmul -> cast -> DRAM store -> diagonal gather
                s = hs[hh]
                ringA = nc.sync if hh == 0 else nc.gpsimd
                ringB = nc.gpsimd if hh == 0 else nc.sync
                (q0, qn) = QCHUNKS[ci]
                tp = ps_s.tile([128, 3, 512], F32, tag="s", name=f"tph{hh}_{ci}")
                nc.tensor.matmul(
                    tp[0:95, 0, 0:qn], rhv_sb[64:128, :],
                    s["rq"][64:128, q0 : q0 + qn], start=True, stop=True,
                )
                nc.vector.tensor_copy(s["t1sb"][:, q0 : q0 + qn], tp[0:95, 0, 0:qn])
                ringA.dma_start(
                    t1d[hh][0:95, q0 : q0 + qn], s["t1sb"][0:95, q0 : q0 + qn]
                )
                (ba, bb) = PIECES[ci]
                nbl = bb - ba
                # rel_h: rq[j, (h,w)] = T1h[47-h+j, 48h+w], h in [ba, bb)
                dsth = s["rq"][0:48, 48 * ba : 48 * bb].rearrange(
                    "p (h w) -> p h w", w=48
                )
                srch = t1d[hh][47 - ba : 95 - ba, 48 * ba : 48 * bb].rearrange(
                    "j (h w) -> j h w", w=48
                )
                srch.ap[1] = [-2256, nbl]
                ringB.dma_start(dsth, srch)

            def relw_exp(hh, half):
                # exp with an un-permuting (w-major -> q-major) input AP
                s = hs[hh]
                h0 = half * 24  # h-blocks 0:24/24:48 -> cols 0:1152/1152:2304
                co, cn = h0 * 48, 1152
                nc.scalar.activation(
                    s["expw"][:, co : co + cn].rearrange("p (h w) -> p h w", w=48),
                    s["relw"][:, :].rearrange("p (w h) -> p h w", w=48)[:, h0 : h0 + 24, :],
                    Exp,
                )

            def pats_copy(hh, co, cn):
                # 3 row-rotations (k-tile offsets 0/32/16) of exp_relw
                s = hs[hh]
                ndma = 0
                for j, off in enumerate((0, 32, 16)):
                    p = 0
                    while p < 128:
                        r0 = (p + off) % 48
                        n = min(48 - r0, 128 - p)
                        eng = (nc.sync, nc.gpsimd)[(hh + ndma) % 2]
                        eng.dma_start(
                            s["pats"][p : p + n, j, co : co + cn],
                            s["expw"][r0 : r0 + n, co : co + cn],
                        )
                        p += n
                        ndma += 1

            # ---- main loop machinery (per head) ----
            def make_main(hh):
                s = hs[hh]
                st = {"pend": [], "epiB": [], "g": 0}

                def epilogue_a(ci, q0, qn, o_ps):
                    rect = epool.tile([1, 512], F32, tag="rect", name=f"rect{hh}_{ci}")
                    nc.vector.reciprocal(rect[0:1, 0:qn], o_ps[64:65, 0:qn])
                    recr = epool.tile([1, 512], BF16, tag="recr", name=f"recr{hh}_{ci}")
                    nc.vector.tensor_copy(recr[0:1, 0:qn], rect[0:1, 0:qn])
                    return recr

                def epilogue_b(ci, q0, qn, o_ps, recr):
                    rb_ps = ps_s.tile([128, 3, 512], F32, tag="s", name=f"rb{hh}_{ci}")
                    nc.tensor.matmul(
                        rb_ps[0:64, 0, 0:qn], ones1[:], recr[0:1, 0:qn],
                        start=True, stop=True,
                    )
                    rb_sb = epool.tile([64, 512], F32, tag="rb_sb", name=f"rbs{hh}_{ci}")
                    nc.vector.tensor_copy(rb_sb[:, 0:qn], rb_ps[0:64, 0, 0:qn])
                    ot = epool.tile([64, 512], F32, tag="ot", name=f"ot{hh}_{ci}")
                    nc.vector.tensor_mul(
                        ot[:, 0:qn], o_ps[0:64, 0:qn], rb_sb[:, 0:qn]
                    )
                    dmae = nc.sync if ci % 2 == 0 else nc.gpsimd
                    dmae.dma_start(out_t[s["c0"] : s["c1"], q0 : q0 + qn], ot[:, 0:qn])

                def flush(last_mm, keep):
                    while len(st["pend"]) > keep:
                        (ci, q0, qn, o_ps, g, p2) = st["pend"].pop(0)
                        for j in range(3):
                            kt = 3 * g + j
                            pv = nc.tensor.matmul(
                                o_ps[0:65, 0:qn],
                                s["vt"][:, kt * 65 : (kt + 1) * 65],
                                p2[:, j, 0:qn],
                                start=(kt == 0), stop=(kt == KT - 1),
                            )
                            if last_mm is not None:
                                add_dep_helper(pv.ins, last_mm.ins, sync=False,
                                               reason="pv after score mms")
                        if g == NG - 1:
                            recr = epilogue_a(ci, q0, qn, o_ps)
                            st["epiB"].append((ci, q0, qn, o_ps, recr, st["g"] + RB_LAG))
                    while st["epiB"] and st["epiB"][0][5] <= st["g"]:
                        (ci, q0, qn, o_ps, recr, _) = st["epiB"].pop(0)
                        epilogue_b(ci, q0, qn, o_ps, recr)

                def chunk(ci, inserts=None):
                    (q0, qn) = QCHUNKS[ci]
                    o_ps = ps_o.tile([65, 512], F32, tag="o", name=f"o{hh}_{ci}")
                    for g in range(NG):
                        if inserts:
                            inserts.pop(0)()
                        s_ps = ps_s.tile([128, 3, 512], F32, tag="s",
                                         name=f"s{hh}_{ci}_{g}")
                        last_mm = None
                        for j in range(3):
                            kt = 3 * g + j
                            last_mm = nc.tensor.matmul(
                                s_ps[:, j, 0:qn],
                                s["lhsT"][:, kt * 128 : (kt + 1) * 128],
                                s["rq"][:, q0 : q0 + qn],
                                start=True, stop=True,
                            )
                        st["g"] += 1
                        flush(last_mm, PV_LAG - 1)
                        p1 = p1pool.tile([128, 3, 512], BF16, tag="p1")
                        nc.scalar.activation(p1[:, :, 0:qn], s_ps[:, :, 0:qn], Exp)
                        p2 = p2pool.tile([128, 3, 512], BF16, tag="p2")
                        nc.vector.tensor_mul(
                            p2[:, :, 0:qn], p1[:, :, 0:qn],
                            s["pats"][:, :, q0 : q0 + qn],
                        )
                        st["pend"].append((ci, q0, qn, o_ps, g, p2))

                def finish_pvs():
                    st["g"] += PV_LAG
                    flush(None, 0)

                def drain(n=1):
                    for _ in range(n):
                        if st["epiB"]:
                            (ci, q0, qn, o_ps, recr, _) = st["epiB"].pop(0)
                            epilogue_b(ci, q0, qn, o_ps, recr)

                return chunk, finish_pvs, drain

            # ---- schedule ----
            ph1_load(0)
            ph1_load(1)
            # critical chain to chunk 0: all T2w -> exp half0 -> pats half0,
            # plus T1h pieces 0-1 (rel_h rows for cols 0:1008)
            for c in range(5):
                tabw(0, c)
            tabh(0, 0)
            tabh(0, 1)
            relw_exp(0, 0)
            pats_copy(0, 0, 1152)

            chunk0, finish_pvs0, drain0 = make_main(0)
            chunk1, finish_pvs1, drain1 = make_main(1)

            nop = lambda: None
            ins0 = [
                # during head-0 chunk 0 (6 slots)
                lambda: tabh(0, 2),
                lambda: tabw(1, 0),
                lambda: tabw(1, 1),
                nop,
                lambda: relw_exp(0, 1),
                lambda: tabw(1, 2),
                # chunk 1
                lambda: tabw(1, 3),
                lambda: pats_copy(0, 1152, 1152),
                lambda: tabw(1, 4),
                nop,
                lambda: tabh(0, 3),
                nop,
                # chunk 2
                lambda: tabh(0, 4),
                nop,
                lambda: relw_exp(1, 0),
                nop,
                lambda: pats_copy(1, 0, 1152),
                nop,
                # chunk 3
                lambda: tabh(1, 0),
                nop,
                lambda: tabh(1, 1),
                nop,
                lambda: relw_exp(1, 1),
                nop,
                # chunk 4
                lambda: tabh(1, 2),
                nop,
                nop,
                nop,
                nop,
                nop,
            ]
            ins1 = [
                # during head-1 chunk 0
                lambda: pats_copy(1, 1152, 1152),
                nop,
                lambda: tabh(1, 3),
                nop,
                lambda: tabh(1, 4),
                nop,
                # chunk 1
                lambda: drain0(1),
                nop,
                lambda: drain0(1),
                nop,
                nop,
                nop,
            ] + [nop] * 18

            chunk0(0, ins0)
            chunk0(1, ins0)
            chunk0(2, ins0)
            chunk0(3, ins0)
            chunk0(4, ins0)
            finish_pvs0()
            chunk1(0, ins1)
            drain0(2)
            chunk1(1, ins1)
            chunk1(2, ins1)
            chunk1(3, ins1)
            chunk1(4, ins1)
            finish_pvs1()
            drain1(5)

    nc.compile()
    return nc


def _get_nc():
    global _NC
    if _NC is None:
        _NC = _build_nc()
    return _NC


def _host_prep(q, k, v, rel_pos_h, rel_pos_w):
    q2 = np.asarray(q, np.float32).reshape(HW, NH * DH)
    k2 = np.asarray(k, np.float32).reshape(HW, NH * DH)
    v2 = np.asarray(v, np.float32).reshape(HW, NH * DH)
    rph = np.asarray(rel_pos_h, np.float32)
    rpw = np.asarray(rel_pos_w, np.float32)

    ar = np.arange(48)
    # reversed rel tables, x8 cancels the 0.125 q scale
    rhv = np.ascontiguousarray((8.0 * rph[::-1]).T).astype(BF)   # (64, 95)
    rwv = np.ascontiguousarray((8.0 * rpw[::-1]).T).astype(BF)
    kk = np.arange(HW)
    eh = np.zeros((64, HW), np.float32)
    eh[:48] = kk[None, :] // 48 == ar[:, None]
    eh = eh.astype(BF)

    onecol = np.ones((HW, 1), np.float32)
    in_maps = []
    for c in range(N_CORES):
        sl = slice(c * 128, (c + 1) * 128)
        qs = (q2[:, sl].T * 0.125).astype(BF)                    # (128, HW)
        qw = np.ascontiguousarray(
            qs.reshape(128, 48, 48).transpose(0, 2, 1)
        ).reshape(128, HW)                                       # w-major cols
        ks = k2[:, sl].T.astype(BF)
        lhs_p = np.zeros((128, HPC, HW), BF)
        rqs_p = np.zeros((80, HPC, HW), BF)
        rqw_p = np.zeros((64, HPC, HW), BF)
        vparts = []
        for hh in range(HPC):
            r0, r1 = hh * 64, (hh + 1) * 64
            lhs_p[0:64, hh, :] = eh
            lhs_p[64:128, hh, :] = ks[r0:r1]
            rqs_p[16:80, hh, :] = qs[r0:r1]
            rqw_p[:, hh, :] = qw[r0:r1]
            vh = v2[:, c * 128 + hh * 64 : c * 128 + (hh + 1) * 64]
            va = np.concatenate([vh, onecol], axis=1)            # (HW, 65)
            vparts.append(va.reshape(KT, 128, 65).transpose(1, 0, 2).reshape(128, KT * 65))
        v_til = np.concatenate(vparts, axis=1).astype(BF)        # (128, 2*18*65)
        in_maps.append(
            dict(
                lhs_p=lhs_p.reshape(128, HPC * HW),
                rqs_p=rqs_p.reshape(80, HPC * HW),
                rqw_p=rqw_p.reshape(64, HPC * HW),
                v_til=v_til, rhv=rhv, rwv=rwv,
            )
        )
    return in_maps


def _assemble(results):
    cols = [np.asarray(r["out_t"], np.float32).T for r in results]  # (HW, 128) each
    return np.concatenate(cols, axis=1).reshape(1, H, W, NH * DH)


def kernel(q, k, v, rel_pos_h, rel_pos_w):
    nc = _get_nc()
    in_maps = _host_prep(q, k, v, rel_pos_h, rel_pos_w)
    res = run_bass_kernel_spmd(nc, in_maps, core_ids=list(range(N_CORES)))
    return _assemble(res.results)
